# revision 10
# baseline (speedup 1.0000x reference)
"""Multi-head attention (B=2, S=2048, D=1024, H=16) on 8 trn2 NeuronCores.

Sharding: batch x head-group. Core c handles batch c//4 and heads
(c%4)*4 .. (c%4)*4+4 (tensor-parallel column split of W_q/W_k/W_v, row
split of W_o). No on-device collectives: the W_o row-split partial sums
(4 per batch) are reduced on the host, and the per-head attention
matrices are gathered on the host.

Activations/weights are uploaded pre-cast to bf16 (matmul operand
precision on trn2); accumulation stays fp32 in PSUM. The attention
matrix is written to DRAM in bf16 [k, q] layout and upcast/transposed
on the host.

Device program (SPMD, one NEFF, different per-core data):
  phase 1: PE-transpose q/k/v tiles (bf16, via identity matmul), then
           project: qT/kT [d', s] head-packed; v natural [s, d'] with
           a ones column appended (softmax denominator comes out of the
           ctx matmul for free).
  phase 2 (qw outer, h inner): scoresT = kT^T qT -> exp (ACT) -> eT;
           dense ctx-accumulation burst on PE (keeps HAM unthrottled);
           1/Z via DVE reciprocal, GPSIMD partition-broadcast; eT
           normalized (bf16 2x DVE) and DMA'd out; after each qw ring
           of 4 heads, the output-projection slice for that s-window
           runs immediately (keeps PE dense, no phase barrier).
"""

import math
from contextlib import ExitStack

import numpy as np
import ml_dtypes

import concourse.bass as bass
import concourse.mybir as mybir
import concourse.tile as tile
from concourse import bacc
from concourse import bass_utils

F32 = mybir.dt.float32
BF16 = mybir.dt.bfloat16
AF = mybir.ActivationFunctionType

B = 2
S = 2048
D = 1024
H = 16
DK = 64
HPC = 4          # heads per core
DSL = HPC * DK   # 256, per-core d' slice
N_CORES = 8
SCALE = 1.0 / math.sqrt(DK)

_CACHED_NC = None


def build_nc():
    nc = bacc.Bacc(trn_type="TRN2", target_bir_lowering=False, debug=False)

    xq = nc.dram_tensor("xq", [S, D], BF16, kind="ExternalInput").ap()
    xk = nc.dram_tensor("xk", [S, D], BF16, kind="ExternalInput").ap()
    xv = nc.dram_tensor("xv", [S, D], BF16, kind="ExternalInput").ap()
    wq = nc.dram_tensor("wq", [D, DSL], BF16, kind="ExternalInput").ap()
    wk = nc.dram_tensor("wk", [D, DSL], BF16, kind="ExternalInput").ap()
    wv = nc.dram_tensor("wv", [D, DSL], BF16, kind="ExternalInput").ap()
    wo = nc.dram_tensor("wo", [DSL, D], BF16, kind="ExternalInput").ap()
    bq = nc.dram_tensor("bq", [DSL], F32, kind="ExternalInput").ap()
    bk = nc.dram_tensor("bk", [DSL], F32, kind="ExternalInput").ap()
    bv = nc.dram_tensor("bv", [DSL], F32, kind="ExternalInput").ap()

    attn_o = nc.dram_tensor("attn", [HPC, S, S], BF16, kind="ExternalOutput").ap()
    out_o = nc.dram_tensor("outp", [D, S], F32, kind="ExternalOutput").ap()

    from concourse.masks import make_identity

    with tile.TileContext(nc) as tc:
        with ExitStack() as top:
            pers = top.enter_context(tc.tile_pool(name="pers", bufs=1))
            # qT/kT: [d', s]; head h at partitions 64*(h%2).., free block (h//2)*S
            qT = pers.tile([128, 2 * S], BF16, name="qT")
            kT = pers.tile([128, 2 * S], BF16, name="kT")
            # v natural [s, d'] per (s-tile, head): [128, 65], col 64 = 1.0
            vaug = pers.tile([128, 16 * HPC * 65], BF16, name="vaug")
            # normalized ctx^T, head h at free block h*S, partitions 0:64
            ctxT = pers.tile([128, HPC * S], BF16, name="ctxT")
            wo_sb = pers.tile([64, HPC * D], BF16, name="wo_sb")
            wq_sb = pers.tile([128, 8 * DSL], BF16, name="wq_sb")
            wk_sb = pers.tile([128, 8 * DSL], BF16, name="wk_sb")
            wv_sb = pers.tile([128, 8 * DSL], BF16, name="wv_sb")
            ident = pers.tile([128, 128], BF16, name="ident")
            bq_sb = pers.tile([128, 2], F32, name="bq_sb")
            bk_sb = pers.tile([128, 2], F32, name="bk_sb")
            bv_st = pers.tile([1, DSL], F32, name="bv_st")
            bv_bc = pers.tile([128, DSL], F32, name="bv_bc")

            make_identity(nc, ident)
            vview = vaug[:].rearrange("p (t c) -> p t c", c=65)
            nc.vector.memset(vview[:, :, 64:65], 1.0)

            nc.sync.dma_start(bq_sb[:], bq.rearrange("(t p) -> p t", p=128))
            nc.sync.dma_start(bk_sb[:], bk.rearrange("(t p) -> p t", p=128))
            nc.sync.dma_start(bv_st[0:1, :], bv.unsqueeze(0))
            nc.gpsimd.partition_broadcast(bv_bc[:], bv_st[0:1, :])
            for dst, src in ((wq_sb, wq), (wk_sb, wk), (wv_sb, wv)):
                nc.sync.dma_start(
                    dst[:].rearrange("p (c d) -> p c d", c=8),
                    src.rearrange("(c p) d -> p c d", p=128))
            nc.sync.dma_start(
                wo_sb[0:64, :].rearrange("e (j d) -> e j d", j=HPC),
                wo.rearrange("(j e) d -> e j d", e=DK))

            # shared PSUM pools (8 banks total, no phase scoping)
            tpsum = top.enter_context(
                tc.tile_pool(name="tpsum", bufs=1, space="PSUM"))   # 1 bank
            ppsum = top.enter_context(
                tc.tile_pool(name="ppsum", bufs=2, space="PSUM"))   # 2 banks
            spsum = top.enter_context(
                tc.tile_pool(name="spsum", bufs=2, space="PSUM"))   # 4 banks
            cpsum = top.enter_context(
                tc.tile_pool(name="cpsum", bufs=1, space="PSUM"))   # 1 bank

            natp = top.enter_context(tc.tile_pool(name="natp", bufs=2))
            actp = top.enter_context(tc.tile_pool(name="actp", bufs=1))
            etp = top.enter_context(tc.tile_pool(name="etp", bufs=20))
            normp = top.enter_context(tc.tile_pool(name="normp", bufs=6))
            rbp = top.enter_context(tc.tile_pool(name="rbp", bufs=2))
            rzp = top.enter_context(tc.tile_pool(name="rzp", bufs=2))
            ostp = top.enter_context(tc.tile_pool(name="ostp", bufs=3))

            # ---------------- phase 1: transpose + project ----------------
            for which, x_ap in (("q", xq), ("k", xk), ("v", xv)):
                actT = actp.tile([128, 8 * S], BF16, tag="actT",
                                 name=f"actT_{which}")
                for stg in range(4):
                    nats = []
                    for g in range(4):
                        st = stg * 4 + g
                        nat = natp.tile([128, D], BF16, tag=f"nat{g}",
                                        name=f"nat_{which}_{st}")
                        nc.sync.dma_start(
                            nat[:], x_ap[st * 128:(st + 1) * 128, :])
                        nats.append(nat)
                    for c in range(8):
                        ps = tpsum.tile([128, 512], BF16, tag="tps",
                                        name=f"tps_{which}_{stg}_{c}")
                        for g in range(4):
                            nc.tensor.transpose(
                                ps[:, g * 128:(g + 1) * 128],
                                nats[g][:, c * 128:(c + 1) * 128],
                                ident[:])
                        nc.vector.tensor_copy(
                            actT[:, c * S + stg * 512: c * S + (stg + 1) * 512],
                            ps[:])

                if which in ("q", "k"):
                    wsb = wq_sb if which == "q" else wk_sb
                    dst = qT if which == "q" else kT
                    bsb = bq_sb if which == "q" else bk_sb
                    for sc in range(4):
                        for t in range(2):
                            ps = ppsum.tile([128, 512], F32, tag="pps",
                                            name=f"pps_{which}_{sc}_{t}")
                            for c in range(8):
                                nc.tensor.matmul(
                                    ps[:],
                                    lhsT=wsb[:, c * DSL + t * 128:
                                             c * DSL + (t + 1) * 128],
                                    rhs=actT[:, c * S + sc * 512:
                                             c * S + (sc + 1) * 512],
                                    start=(c == 0), stop=(c == 7))
                            nc.scalar.activation(
                                dst[:, t * S + sc * 512: t * S + (sc + 1) * 512],
                                ps[:], AF.Identity,
                                bias=bsb[:, t:t + 1], scale=1.0)
                else:
                    for st in range(16):
                        ps = ppsum.tile([128, 256], F32, tag="pps",
                                        name=f"pps_v_{st}")
                        for c in range(8):
                            nc.tensor.matmul(
                                ps[:],
                                lhsT=actT[:, c * S + st * 128:
                                          c * S + (st + 1) * 128],
                                rhs=wv_sb[:, c * DSL:(c + 1) * DSL],
                                start=(c == 0), stop=(c == 7))
                        for j in range(HPC):
                            off = (st * HPC + j) * 65
                            nc.vector.tensor_add(
                                vaug[:, off:off + 64],
                                ps[:, j * 64:(j + 1) * 64],
                                bv_bc[:, j * 64:(j + 1) * 64])

            # -------- phase 2: attention (+ fused output projection) --------
            for qw in range(4):
                for h in range(HPC):
                    p0 = 64 * (h % 2)
                    blk = (h // 2) * S
                    q0 = blk + qw * 512
                    # loop 1: scores -> exp pipeline
                    ets = []
                    for pr in range(8):
                        sp = spsum.tile([128, 1024], F32, tag="sc",
                                        name=f"sp_{h}_{qw}_{pr}")
                        for hf in range(2):
                            kt = pr * 2 + hf
                            nc.tensor.matmul(
                                sp[:, hf * 512:(hf + 1) * 512],
                                lhsT=kT[p0:p0 + 64,
                                        blk + kt * 128: blk + (kt + 1) * 128],
                                rhs=qT[p0:p0 + 64, q0:q0 + 512])
                        et = etp.tile([128, 1024], BF16, tag="et",
                                      name=f"et_{h}_{qw}_{pr}")
                        nc.scalar.activation(et[:], sp[:], AF.Exp,
                                             bias=0.0, scale=SCALE)
                        ets.append(et)
                    # loop 2: ctx accumulation — dense PE burst
                    ctx_ps = cpsum.tile([65, 512], F32, tag="ctx",
                                        name=f"ctx_{h}_{qw}")
                    for kt in range(16):
                        nc.tensor.matmul(
                            ctx_ps[:],
                            lhsT=vaug[:, (kt * HPC + h) * 65:
                                      (kt * HPC + h) * 65 + 65],
                            rhs=ets[kt // 2][:, (kt % 2) * 512:
                                             (kt % 2 + 1) * 512],
                            start=(kt == 0), stop=(kt == 15),
                            skip_group_check=True)

                    rz = rzp.tile([1, 512], F32, tag="rz",
                                  name=f"rz_{h}_{qw}")
                    nc.vector.reciprocal(rz[:], ctx_ps[64:65, :])
                    rz16 = rzp.tile([1, 512], BF16, tag="rz16",
                                    name=f"rz16_{h}_{qw}")
                    nc.vector.tensor_copy(rz16[:], rz[:])
                    rb = rbp.tile([128, 512], BF16, tag="rb",
                                  name=f"rb_{h}_{qw}")
                    nc.gpsimd.partition_broadcast(rb[:], rz16[0:1, :])
                    nc.vector.tensor_mul(
                        ctxT[0:64, h * S + qw * 512: h * S + qw * 512 + 512],
                        ctx_ps[0:64, :], rb[0:64, :])

                    rb3 = rb[:].unsqueeze(1).broadcast_to((128, 2, 512))
                    for pr in range(8):
                        nt = normp.tile([128, 1024], BF16, tag="nt",
                                        name=f"nt_{h}_{qw}_{pr}")
                        nc.vector.tensor_mul(
                            nt[:].rearrange("p (a q) -> p a q", a=2),
                            ets[pr][:].rearrange("p (a q) -> p a q", a=2),
                            rb3)
                        dst = attn_o[h,
                                     pr * 256:(pr + 1) * 256,
                                     qw * 512:(qw + 1) * 512]
                        nc.sync.dma_start(
                            dst.rearrange("(a p) q -> p a q", p=128),
                            nt[:].rearrange("p (a q) -> p a q", a=2))

                # output projection for this s-window (sc = qw)
                sc = qw
                for dt in range(8):
                    ps = ppsum.tile([128, 512], F32, tag="pps",
                                    name=f"ops_{dt}_{sc}")
                    for j in range(HPC):
                        nc.tensor.matmul(
                            ps[:],
                            lhsT=wo_sb[0:64, j * D + dt * 128:
                                       j * D + (dt + 1) * 128],
                            rhs=ctxT[0:64, j * S + sc * 512:
                                     j * S + (sc + 1) * 512],
                            start=(j == 0), stop=(j == HPC - 1))
                    os_ = ostp.tile([128, 512], F32, tag="os",
                                    name=f"os_{dt}_{sc}")
                    nc.scalar.copy(os_[:], ps[:])
                    nc.sync.dma_start(
                        out_o[dt * 128:(dt + 1) * 128,
                              sc * 512:(sc + 1) * 512], os_[:])

    nc.compile()
    return nc


def get_nc():
    global _CACHED_NC
    if _CACHED_NC is None:
        _CACHED_NC = build_nc()
    return _CACHED_NC


def make_in_maps(inputs):
    bf = ml_dtypes.bfloat16
    q = np.asarray(inputs["query"], dtype=np.float32)
    k = np.asarray(inputs["key"], dtype=np.float32)
    v = np.asarray(inputs["value"], dtype=np.float32)
    in_maps = []
    for c in range(N_CORES):
        b = c // 4
        cs = (c % 4) * DSL
        in_maps.append({
            "xq": np.ascontiguousarray(q[b]).astype(bf),
            "xk": np.ascontiguousarray(k[b]).astype(bf),
            "xv": np.ascontiguousarray(v[b]).astype(bf),
            "wq": np.ascontiguousarray(np.asarray(inputs["W_q"], np.float32)[:, cs:cs + DSL]).astype(bf),
            "wk": np.ascontiguousarray(np.asarray(inputs["W_k"], np.float32)[:, cs:cs + DSL]).astype(bf),
            "wv": np.ascontiguousarray(np.asarray(inputs["W_v"], np.float32)[:, cs:cs + DSL]).astype(bf),
            "wo": np.ascontiguousarray(np.asarray(inputs["W_o"], np.float32)[cs:cs + DSL, :]).astype(bf),
            "bq": np.ascontiguousarray(np.asarray(inputs["b_q"], np.float32)[cs:cs + DSL]),
            "bk": np.ascontiguousarray(np.asarray(inputs["b_k"], np.float32)[cs:cs + DSL]),
            "bv": np.ascontiguousarray(np.asarray(inputs["b_v"], np.float32)[cs:cs + DSL]),
        })
    return in_maps


def gather_outputs(results, b_o):
    b_o = np.asarray(b_o, np.float32)
    out = np.empty((B, S, D), np.float32)
    attn_t = np.empty((B, H, S, S), np.float32)  # [b, h, k, q]
    for b in range(B):
        acc = None
        for g in range(4):
            c = b * 4 + g
            part = results[c]["outp"]  # [D, S] fp32
            acc = part if acc is None else acc + part
            attn_t[b, g * HPC:(g + 1) * HPC] = results[c]["attn"].astype(np.float32)
        out[b] = acc.T + b_o
    attn = attn_t.transpose(0, 1, 3, 2)  # view: [b, h, q, k]
    return out, attn


def run(inputs, trace=False):
    nc = get_nc()
    in_maps = make_in_maps(inputs)
    res = bass_utils.run_bass_kernel_spmd(
        nc, in_maps, core_ids=list(range(N_CORES)), trace=trace)
    out, attn = gather_outputs(res.results, inputs["b_o"])
    return (out, attn), res


def kernel(**inputs):
    (out, attn), _ = run(inputs, trace=False)
    return out, attn


# revision 11
# speedup vs baseline: 1.1097x; 1.1097x over previous
"""Multi-head attention (B=2, S=2048, D=1024, H=16) on 8 trn2 NeuronCores.

Sharding: batch x head-group. Core c handles batch c//4 and heads
(c%4)*4 .. (c%4)*4+4 (tensor-parallel column split of W_q/W_k/W_v, row
split of W_o). No on-device collectives: the W_o row-split partial sums
(4 per batch) are reduced on the host, and the per-head attention
matrices are gathered on the host.

Activations/weights are uploaded pre-cast to bf16 (matmul operand
precision on trn2); accumulation stays fp32 in PSUM. The attention
matrix is written to DRAM in bf16 [k, q] layout and upcast/transposed
on the host.

Device program (SPMD, one NEFF, different per-core data):
  phase 1: PE-transpose q/k/v tiles (bf16, via identity matmul), then
           project: qT/kT [d', s] head-packed; v natural [s, d'] with
           a ones column appended (softmax denominator comes out of the
           ctx matmul for free).
  phase 2 (qw outer, h inner): scoresT = kT^T qT -> exp (ACT) -> eT;
           dense ctx-accumulation burst on PE (keeps HAM unthrottled);
           1/Z via DVE reciprocal, GPSIMD partition-broadcast; eT
           normalized (bf16 2x DVE) and DMA'd out; after each qw ring
           of 4 heads, the output-projection slice for that s-window
           runs immediately (keeps PE dense, no phase barrier).
"""

import math
from contextlib import ExitStack

import numpy as np
import ml_dtypes

import concourse.bass as bass
import concourse.mybir as mybir
import concourse.tile as tile
from concourse import bacc
from concourse import bass_utils

F32 = mybir.dt.float32
BF16 = mybir.dt.bfloat16
AF = mybir.ActivationFunctionType

B = 2
S = 2048
D = 1024
H = 16
DK = 64
HPC = 4          # heads per core
DSL = HPC * DK   # 256, per-core d' slice
N_CORES = 8
SCALE = 1.0 / math.sqrt(DK)

_CACHED_NC = None


def build_nc():
    nc = bacc.Bacc(trn_type="TRN2", target_bir_lowering=False, debug=False)

    xq = nc.dram_tensor("xq", [S, D], BF16, kind="ExternalInput").ap()
    xk = nc.dram_tensor("xk", [S, D], BF16, kind="ExternalInput").ap()
    xv = nc.dram_tensor("xv", [S, D], BF16, kind="ExternalInput").ap()
    wq = nc.dram_tensor("wq", [D, DSL], BF16, kind="ExternalInput").ap()
    wk = nc.dram_tensor("wk", [D, DSL], BF16, kind="ExternalInput").ap()
    wv = nc.dram_tensor("wv", [D, DSL], BF16, kind="ExternalInput").ap()
    wo = nc.dram_tensor("wo", [DSL, D], BF16, kind="ExternalInput").ap()
    bq = nc.dram_tensor("bq", [DSL], F32, kind="ExternalInput").ap()
    bk = nc.dram_tensor("bk", [DSL], F32, kind="ExternalInput").ap()
    bv = nc.dram_tensor("bv", [DSL], F32, kind="ExternalInput").ap()

    attn_o = nc.dram_tensor("attn", [HPC, S, S], BF16, kind="ExternalOutput").ap()
    out_o = nc.dram_tensor("outp", [D, S], F32, kind="ExternalOutput").ap()

    from concourse.masks import make_identity

    with tile.TileContext(nc) as tc:
        with ExitStack() as top:
            pers = top.enter_context(tc.tile_pool(name="pers", bufs=1))
            # qT/kT: [d', s]; head h at partitions 64*(h%2).., free block (h//2)*S
            qT = pers.tile([128, 2 * S], BF16, name="qT")
            kT = pers.tile([128, 2 * S], BF16, name="kT")
            # v natural [s, d'] per (s-tile, head): [128, 65], col 64 = 1.0
            vaug = pers.tile([128, 16 * HPC * 65], BF16, name="vaug")
            # normalized ctx^T, head h at free block h*S, partitions 0:64
            ctxT = pers.tile([128, HPC * S], BF16, name="ctxT")
            wo_sb = pers.tile([64, HPC * D], BF16, name="wo_sb")
            wq_sb = pers.tile([128, 8 * DSL], BF16, name="wq_sb")
            wk_sb = pers.tile([128, 8 * DSL], BF16, name="wk_sb")
            wv_sb = pers.tile([128, 8 * DSL], BF16, name="wv_sb")
            ident = pers.tile([128, 128], BF16, name="ident")
            bq_sb = pers.tile([128, 2], F32, name="bq_sb")
            bk_sb = pers.tile([128, 2], F32, name="bk_sb")
            bv_st = pers.tile([1, DSL], F32, name="bv_st")
            bv_bc = pers.tile([128, DSL], F32, name="bv_bc")

            make_identity(nc, ident)
            vview = vaug[:].rearrange("p (t c) -> p t c", c=65)
            nc.vector.memset(vview[:, :, 64:65], 1.0)

            nc.sync.dma_start(bq_sb[:], bq.rearrange("(t p) -> p t", p=128))
            nc.sync.dma_start(bk_sb[:], bk.rearrange("(t p) -> p t", p=128))
            nc.sync.dma_start(bv_st[0:1, :], bv.unsqueeze(0))
            nc.gpsimd.partition_broadcast(bv_bc[:], bv_st[0:1, :])
            for dst, src in ((wq_sb, wq), (wk_sb, wk), (wv_sb, wv)):
                nc.scalar.dma_start(
                    dst[:].rearrange("p (c d) -> p c d", c=8),
                    src.rearrange("(c p) d -> p c d", p=128))
            nc.scalar.dma_start(
                wo_sb[0:64, :].rearrange("e (j d) -> e j d", j=HPC),
                wo.rearrange("(j e) d -> e j d", e=DK))

            # shared PSUM pools (8 banks total, no phase scoping)
            tpsum = top.enter_context(
                tc.tile_pool(name="tpsum", bufs=1, space="PSUM"))   # 1 bank
            ppsum = top.enter_context(
                tc.tile_pool(name="ppsum", bufs=2, space="PSUM"))   # 2 banks
            spsum = top.enter_context(
                tc.tile_pool(name="spsum", bufs=2, space="PSUM"))   # 4 banks
            cpsum = top.enter_context(
                tc.tile_pool(name="cpsum", bufs=1, space="PSUM"))   # 1 bank

            natp = top.enter_context(tc.tile_pool(name="natp", bufs=2))
            actp = top.enter_context(tc.tile_pool(name="actp", bufs=1))
            etp = top.enter_context(tc.tile_pool(name="etp", bufs=20))
            normp = top.enter_context(tc.tile_pool(name="normp", bufs=6))
            rbp = top.enter_context(tc.tile_pool(name="rbp", bufs=2))
            rzp = top.enter_context(tc.tile_pool(name="rzp", bufs=2))
            ostp = top.enter_context(tc.tile_pool(name="ostp", bufs=3))
            csbp = top.enter_context(tc.tile_pool(name="csbp", bufs=2))

            # ---------------- phase 1: transpose + project ----------------
            for which, x_ap in (("q", xq), ("k", xk), ("v", xv)):
                actT = actp.tile([128, 8 * S], BF16, tag="actT",
                                 name=f"actT_{which}")
                for stg in range(4):
                    nats = []
                    for g in range(4):
                        st = stg * 4 + g
                        nat = natp.tile([128, D], BF16, tag=f"nat{g}",
                                        name=f"nat_{which}_{st}")
                        nc.sync.dma_start(
                            nat[:], x_ap[st * 128:(st + 1) * 128, :])
                        nats.append(nat)
                    for c in range(8):
                        ps = tpsum.tile([128, 512], BF16, tag="tps",
                                        name=f"tps_{which}_{stg}_{c}")
                        for g in range(4):
                            nc.tensor.transpose(
                                ps[:, g * 128:(g + 1) * 128],
                                nats[g][:, c * 128:(c + 1) * 128],
                                ident[:])
                        nc.vector.tensor_copy(
                            actT[:, c * S + stg * 512: c * S + (stg + 1) * 512],
                            ps[:])

                if which in ("q", "k"):
                    wsb = wq_sb if which == "q" else wk_sb
                    dst = qT if which == "q" else kT
                    bsb = bq_sb if which == "q" else bk_sb
                    for sc in range(4):
                        for t in range(2):
                            ps = ppsum.tile([128, 512], F32, tag="pps",
                                            name=f"pps_{which}_{sc}_{t}")
                            for c in range(8):
                                nc.tensor.matmul(
                                    ps[:],
                                    lhsT=wsb[:, c * DSL + t * 128:
                                             c * DSL + (t + 1) * 128],
                                    rhs=actT[:, c * S + sc * 512:
                                             c * S + (sc + 1) * 512],
                                    start=(c == 0), stop=(c == 7))
                            nc.scalar.activation(
                                dst[:, t * S + sc * 512: t * S + (sc + 1) * 512],
                                ps[:], AF.Identity,
                                bias=bsb[:, t:t + 1], scale=1.0)
                else:
                    for st in range(16):
                        ps = ppsum.tile([128, 256], F32, tag="pps",
                                        name=f"pps_v_{st}")
                        for c in range(8):
                            nc.tensor.matmul(
                                ps[:],
                                lhsT=actT[:, c * S + st * 128:
                                          c * S + (st + 1) * 128],
                                rhs=wv_sb[:, c * DSL:(c + 1) * DSL],
                                start=(c == 0), stop=(c == 7))
                        for j in range(HPC):
                            off = (st * HPC + j) * 65
                            nc.vector.tensor_add(
                                vaug[:, off:off + 64],
                                ps[:, j * 64:(j + 1) * 64],
                                bv_bc[:, j * 64:(j + 1) * 64])

            # -------- phase 2: attention (+ fused output projection) --------
            def emit_outproj(sc):
                for dt in range(8):
                    ps = ppsum.tile([128, 512], F32, tag="pps",
                                    name=f"ops_{dt}_{sc}")
                    for j in range(HPC):
                        nc.tensor.matmul(
                            ps[:],
                            lhsT=wo_sb[0:64, j * D + dt * 128:
                                       j * D + (dt + 1) * 128],
                            rhs=ctxT[0:64, j * S + sc * 512:
                                     j * S + (sc + 1) * 512],
                            start=(j == 0), stop=(j == HPC - 1))
                    os_ = ostp.tile([128, 512], F32, tag="os",
                                    name=f"os_{dt}_{sc}")
                    nc.scalar.copy(os_[:], ps[:])
                    nc.sync.dma_start(
                        out_o[dt * 128:(dt + 1) * 128,
                              sc * 512:(sc + 1) * 512], os_[:])

            for qw in range(4):
                for h in range(HPC):
                    p0 = 64 * (h % 2)
                    blk = (h // 2) * S
                    q0 = blk + qw * 512
                    # loop 1: scores -> exp pipeline
                    ets = []
                    for pr in range(8):
                        sp = spsum.tile([128, 1024], F32, tag="sc",
                                        name=f"sp_{h}_{qw}_{pr}")
                        for hf in range(2):
                            kt = pr * 2 + hf
                            nc.tensor.matmul(
                                sp[:, hf * 512:(hf + 1) * 512],
                                lhsT=kT[p0:p0 + 64,
                                        blk + kt * 128: blk + (kt + 1) * 128],
                                rhs=qT[p0:p0 + 64, q0:q0 + 512])
                        et = etp.tile([128, 1024], BF16, tag="et",
                                      name=f"et_{h}_{qw}_{pr}")
                        nc.scalar.activation(et[:], sp[:], AF.Exp,
                                             bias=0.0, scale=SCALE)
                        ets.append(et)
                    # loop 2: ctx accumulation — dense PE burst
                    ctx_ps = cpsum.tile([65, 512], F32, tag="ctx",
                                        name=f"ctx_{h}_{qw}")
                    for kt in range(16):
                        nc.tensor.matmul(
                            ctx_ps[:],
                            lhsT=vaug[:, (kt * HPC + h) * 65:
                                      (kt * HPC + h) * 65 + 65],
                            rhs=ets[kt // 2][:, (kt % 2) * 512:
                                             (kt % 2 + 1) * 512],
                            start=(kt == 0), stop=(kt == 15),
                            skip_group_check=True)

                    ctx_sb = csbp.tile([65, 512], F32, tag="csb",
                                       name=f"csb_{h}_{qw}")
                    nc.scalar.copy(ctx_sb[:], ctx_ps[:])
                    rz = rzp.tile([1, 512], F32, tag="rz",
                                  name=f"rz_{h}_{qw}")
                    nc.vector.reciprocal(rz[:], ctx_sb[64:65, :])
                    rz16 = rzp.tile([1, 512], BF16, tag="rz16",
                                    name=f"rz16_{h}_{qw}")
                    nc.vector.tensor_copy(rz16[:], rz[:])
                    rb = rbp.tile([128, 512], BF16, tag="rb",
                                  name=f"rb_{h}_{qw}")
                    nc.gpsimd.partition_broadcast(rb[:], rz16[0:1, :])
                    nc.vector.tensor_mul(
                        ctxT[0:64, h * S + qw * 512: h * S + qw * 512 + 512],
                        ctx_sb[0:64, :], rb[0:64, :])

                    rb3 = rb[:].unsqueeze(1).broadcast_to((128, 2, 512))
                    for pr in range(8):
                        nt = normp.tile([128, 1024], BF16, tag="nt",
                                        name=f"nt_{h}_{qw}_{pr}")
                        nc.vector.tensor_mul(
                            nt[:].rearrange("p (a q) -> p a q", a=2),
                            ets[pr][:].rearrange("p (a q) -> p a q", a=2),
                            rb3)
                        dst = attn_o[h,
                                     pr * 256:(pr + 1) * 256,
                                     qw * 512:(qw + 1) * 512]
                        nc.sync.dma_start(
                            dst.rearrange("(a p) q -> p a q", p=128),
                            nt[:].rearrange("p (a q) -> p a q", a=2))

                    if h == 0 and qw > 0:
                        emit_outproj(qw - 1)

            emit_outproj(3)

    nc.compile()
    return nc


def get_nc():
    global _CACHED_NC
    if _CACHED_NC is None:
        _CACHED_NC = build_nc()
    return _CACHED_NC


def make_in_maps(inputs):
    bf = ml_dtypes.bfloat16
    q = np.asarray(inputs["query"], dtype=np.float32)
    k = np.asarray(inputs["key"], dtype=np.float32)
    v = np.asarray(inputs["value"], dtype=np.float32)
    in_maps = []
    for c in range(N_CORES):
        b = c // 4
        cs = (c % 4) * DSL
        in_maps.append({
            "xq": np.ascontiguousarray(q[b]).astype(bf),
            "xk": np.ascontiguousarray(k[b]).astype(bf),
            "xv": np.ascontiguousarray(v[b]).astype(bf),
            "wq": np.ascontiguousarray(np.asarray(inputs["W_q"], np.float32)[:, cs:cs + DSL]).astype(bf),
            "wk": np.ascontiguousarray(np.asarray(inputs["W_k"], np.float32)[:, cs:cs + DSL]).astype(bf),
            "wv": np.ascontiguousarray(np.asarray(inputs["W_v"], np.float32)[:, cs:cs + DSL]).astype(bf),
            "wo": np.ascontiguousarray(np.asarray(inputs["W_o"], np.float32)[cs:cs + DSL, :]).astype(bf),
            "bq": np.ascontiguousarray(np.asarray(inputs["b_q"], np.float32)[cs:cs + DSL]),
            "bk": np.ascontiguousarray(np.asarray(inputs["b_k"], np.float32)[cs:cs + DSL]),
            "bv": np.ascontiguousarray(np.asarray(inputs["b_v"], np.float32)[cs:cs + DSL]),
        })
    return in_maps


def gather_outputs(results, b_o):
    b_o = np.asarray(b_o, np.float32)
    out = np.empty((B, S, D), np.float32)
    attn_t = np.empty((B, H, S, S), np.float32)  # [b, h, k, q]
    for b in range(B):
        acc = None
        for g in range(4):
            c = b * 4 + g
            part = results[c]["outp"]  # [D, S] fp32
            acc = part if acc is None else acc + part
            attn_t[b, g * HPC:(g + 1) * HPC] = results[c]["attn"].astype(np.float32)
        out[b] = acc.T + b_o
    attn = attn_t.transpose(0, 1, 3, 2)  # view: [b, h, q, k]
    return out, attn


def run(inputs, trace=False):
    nc = get_nc()
    in_maps = make_in_maps(inputs)
    res = bass_utils.run_bass_kernel_spmd(
        nc, in_maps, core_ids=list(range(N_CORES)), trace=trace)
    out, attn = gather_outputs(res.results, inputs["b_o"])
    return (out, attn), res


def kernel(**inputs):
    (out, attn), _ = run(inputs, trace=False)
    return out, attn
